# revision 2
# baseline (speedup 1.0000x reference)
"""KAN-SE (squeeze-excite with 2-layer KAN MLP) Trainium2 kernel.

Full-input contract: kernel(**inputs) takes the complete (32, 512, 64, 64)
batch plus KAN weights, shards the batch across 8 NeuronCores (4 samples
per core, data-parallel, weights replicated), and returns the full output.

Per-core device program (pure SPMD, no collectives):
  for each of 4 samples:
    - load the sample's (512, 4096) pixels as 4 tiles of (128, 4096) f32,
      keep them resident in SBUF
    - per-channel mean via free-dim reduce  -> s (512,)
    - 2-layer KAN on s (B-spline bases via Cox-de-Boor on VectorE,
      einsums as tiny PE matmuls accumulating in PSUM, SiLU/Sigmoid on
      ScalarE) -> per-channel gate (512,)
    - scale the resident tiles by the gate and store

x is read exactly once (SBUF-resident between mean and scale), so HBM
traffic is the 2x minimum: 8 MiB in + 8 MiB out per sample per core.
"""

import numpy as np

# ---- problem constants (hardcoded per contract; do not read spec/reference) ----
B, C, H, W = 32, 512, 64, 64
HIDDEN = 64            # max(16, 512 // 8)
KB = 8                 # GRID_SIZE + SPLINE_ORDER = 5 + 3
NCORES = 8
NS = B // NCORES       # samples per core = 4
NG = C // 128          # channel groups of 128 = 4
HWPIX = H * W          # 4096

# gtab column layout: [G0(12) | -g_i for k=1(10) | g_{i+2} k=1(10)
#                      | -g_i k=2(9) | g_{i+3} k=2(9) | -g_i k=3(8) | g_{i+4} k=3(8)]
_GT_OFF = {"G0": 0, 1: (12, 22), 2: (32, 41), 3: (50, 58)}
_GT_W = 66


def _grid_tables(grid_row: np.ndarray):
    """Build the (128, 66) constant table + per-level reciprocal immediates
    from one row of the (uniform) grid."""
    g = np.asarray(grid_row, np.float64)
    assert g.shape == (12,)
    h = g[1] - g[0]
    tab = np.zeros((_GT_W,), np.float64)
    tab[0:12] = g
    rs = {}
    for k in (1, 2, 3):
        w = 11 - k
        aoff, coff = _GT_OFF[k]
        tab[aoff:aoff + w] = -g[:w]          # -g_i,      i = 0..10-k
        tab[coff:coff + w] = g[k + 1:12]     # g_{i+k+1}, i = 0..10-k
        rs[k] = float(np.float32(1.0 / (k * h)))
    full = np.tile(tab.astype(np.float32)[None, :], (128, 1))
    return np.ascontiguousarray(full), rs


def _host_prep(inputs):
    """Rearrange weights into the SBUF layouts the device program uses."""
    f32 = np.float32
    base_w1 = np.asarray(inputs["base_w1"], f32)      # (64, 512)
    spline_w1 = np.asarray(inputs["spline_w1"], f32)  # (64, 512, 8)
    scaler1 = np.asarray(inputs["scaler1"], f32)      # (64, 512)
    base_w2 = np.asarray(inputs["base_w2"], f32)      # (512, 64)
    spline_w2 = np.asarray(inputs["spline_w2"], f32)  # (512, 64, 8)
    scaler2 = np.asarray(inputs["scaler2"], f32)      # (512, 64)

    # w1t[p, g*64+o] = base_w1[o, 128g+p]
    w1t = base_w1.reshape(HIDDEN, NG, 128).transpose(2, 1, 0).reshape(128, NG * HIDDEN)
    # sw1[p, (g*8+k)*64+o] = (spline_w1*scaler1)[o, 128g+p, k]
    sw1 = (spline_w1 * scaler1[:, :, None]).reshape(HIDDEN, NG, 128, KB)
    sw1 = sw1.transpose(2, 1, 3, 0).reshape(128, NG * KB * HIDDEN)
    # w2t[p, o] = base_w2[o, p]
    w2t = base_w2.T
    # sw2[p, k*512+o] = (spline_w2*scaler2)[o, p, k]
    sw2 = (spline_w2 * scaler2[:, :, None]).transpose(1, 2, 0).reshape(HIDDEN, KB * C)

    gt1, rs1 = _grid_tables(np.asarray(inputs["grid1"], f32)[0])
    gt2, rs2 = _grid_tables(np.asarray(inputs["grid2"], f32)[0])

    tensors = {
        "w1t": np.ascontiguousarray(w1t, f32),
        "sw1": np.ascontiguousarray(sw1, f32),
        "w2t": np.ascontiguousarray(w2t, f32),
        "sw2": np.ascontiguousarray(sw2, f32),
        "gt1": gt1,
        "gt2": gt2,
    }
    return tensors, rs1, rs2


def _emit_bsplines(nc, mybir, pool, gt_sb, x_ap, out_ap, p, rs):
    """Cubic B-spline bases of x (one value per partition) -> out_ap (p, 8).

    Cox-de-Boor on VectorE with per-basis-index grid constants from gt_sb
    and uniform-knot reciprocals rs (immediates).
    """
    f32 = mybir.dt.float32
    Alu = mybir.AluOpType
    ge = pool.tile([128, 12], f32, tag="ge", bufs=4)
    # ge[:, i] = (g_i <= x)
    nc.vector.tensor_scalar(
        out=ge[:p], in0=gt_sb[:p, 0:12], scalar1=x_ap, scalar2=None, op0=Alu.is_le
    )
    bprev = pool.tile([128, 11], f32, tag="b0", bufs=4)
    nc.vector.tensor_tensor(bprev[:p], ge[:p, 0:11], ge[:p, 1:12], Alu.subtract)
    for k in (1, 2, 3):
        w = 11 - k
        aoff, coff = _GT_OFF[k]
        a_t = pool.tile([128, 10], f32, tag="bsA", bufs=4)
        c_t = pool.tile([128, 10], f32, tag="bsC", bufs=4)
        # A = (x - g_i) / (k h);  C = (g_{i+k+1} - x) / (k h)
        nc.vector.tensor_scalar(
            out=a_t[:p, :w], in0=gt_sb[:p, aoff:aoff + w], scalar1=x_ap,
            scalar2=rs[k], op0=Alu.add, op1=Alu.mult,
        )
        nc.vector.tensor_scalar(
            out=c_t[:p, :w], in0=gt_sb[:p, coff:coff + w], scalar1=x_ap,
            scalar2=rs[k], op0=Alu.subtract, op1=Alu.mult,
        )
        if k < 3:
            bnext = pool.tile([128, 10], f32, tag="bn", bufs=4)
            outp = bnext[:p, :w]
        else:
            outp = out_ap
        nc.vector.tensor_tensor(c_t[:p, :w], c_t[:p, :w], bprev[:p, 1:w + 1], Alu.mult)
        nc.vector.tensor_tensor(outp, a_t[:p, :w], bprev[:p, 0:w], Alu.mult)
        nc.vector.tensor_tensor(outp, outp, c_t[:p, :w], Alu.add)
        if k < 3:
            bprev = bnext


def _build_nc(rs1, rs2):
    import concourse.bacc as bacc
    import concourse.bass as bass  # noqa: F401
    import concourse.mybir as mybir
    from concourse.tile import TileContext

    f32 = mybir.dt.float32
    Alu = mybir.AluOpType
    Act = mybir.ActivationFunctionType
    AX = mybir.AxisListType

    # Bacc (not plain Bass): its compile() runs move_matmul_waits_to_ldweights
    # + generate_event_semaphores, which split multi-waits down to the 1-wait-
    # per-instruction TRN2 ISA limit that walrus enforces.
    nc = bacc.Bacc("TRN2", target_bir_lowering=False)
    x_d = nc.declare_dram_parameter("x", [NS, C, H, W], f32, isOutput=False)
    w1t_d = nc.declare_dram_parameter("w1t", [128, NG * HIDDEN], f32, isOutput=False)
    sw1_d = nc.declare_dram_parameter("sw1", [128, NG * KB * HIDDEN], f32, isOutput=False)
    w2t_d = nc.declare_dram_parameter("w2t", [HIDDEN, C], f32, isOutput=False)
    sw2_d = nc.declare_dram_parameter("sw2", [HIDDEN, KB * C], f32, isOutput=False)
    gt1_d = nc.declare_dram_parameter("gt1", [128, _GT_W], f32, isOutput=False)
    gt2_d = nc.declare_dram_parameter("gt2", [128, _GT_W], f32, isOutput=False)
    y_d = nc.declare_dram_parameter("y", [NS, C, H, W], f32, isOutput=True)

    with TileContext(nc) as tc:
        with (
            tc.tile_pool(name="consts", bufs=1) as cpool,
            tc.tile_pool(name="xdata", bufs=2 * NG) as xpool,
            tc.tile_pool(name="small", bufs=3) as spool,
            tc.tile_pool(name="bspl", bufs=1) as bpool,
            tc.tile_pool(name="psum", bufs=2, space="PSUM") as ppool,
        ):
            w1t_sb = cpool.tile([128, NG * HIDDEN], f32)
            nc.sync.dma_start(w1t_sb[:], w1t_d[:, :])
            sw1_sb = cpool.tile([128, NG * KB * HIDDEN], f32)
            nc.sync.dma_start(sw1_sb[:], sw1_d[:, :])
            w2t_sb = cpool.tile([HIDDEN, C], f32)
            nc.sync.dma_start(w2t_sb[:], w2t_d[:, :])
            sw2_sb = cpool.tile([HIDDEN, KB * C], f32)
            nc.sync.dma_start(sw2_sb[:], sw2_d[:, :])
            gt1_sb = cpool.tile([128, _GT_W], f32)
            nc.sync.dma_start(gt1_sb[:], gt1_d[:, :])
            gt2_sb = cpool.tile([128, _GT_W], f32)
            nc.sync.dma_start(gt2_sb[:], gt2_d[:, :])

            # Pre-touch every const tile on VectorE: the DMA-completion wait
            # lands on these throwaway copies, so later DVE consumers (notably
            # TensorScalarPtr ops, whose ISA format has a single wait slot)
            # never need a DMA wait of their own.
            touch = cpool.tile([128, 8], f32)
            for i, ct in enumerate((w1t_sb, sw1_sb, gt1_sb, gt2_sb)):
                nc.vector.tensor_copy(touch[:, i:i + 1], ct[:, 0:1])
            for i, ct in enumerate((w2t_sb, sw2_sb)):
                nc.vector.tensor_copy(touch[:HIDDEN, 4 + i:5 + i], ct[:, 0:1])
            # Same for TensorE: the LDWEIGHTS sub-instruction also has a single
            # wait slot, so absorb each weight tile's DMA wait into a throwaway
            # 1-column matmul before the real accumulation chains.
            pt_ps = ppool.tile([1, 4], f32, tag="pt")
            for i, ct in enumerate((w1t_sb, sw1_sb)):
                nc.tensor.matmul(pt_ps[0:1, i:i + 1], ct[:, 0:1], ct[:, 0:1],
                                 start=True, stop=True)
            for i, ct in enumerate((w2t_sb, sw2_sb)):
                nc.tensor.matmul(pt_ps[0:1, 2 + i:3 + i], ct[:HIDDEN, 0:1],
                                 ct[:HIDDEN, 0:1], start=True, stop=True)

            for n in range(NS):
                # ---- load sample, per-channel sums ----
                sT = spool.tile([128, NG], f32, tag="sT")
                xts = []
                for g in range(NG):
                    xt = xpool.tile([128, HWPIX], f32, tag="xt")
                    src = x_d[n, 128 * g:128 * (g + 1)].rearrange("p h w -> p (h w)")
                    nc.sync.dma_start(xt[:], src)
                    nc.vector.reduce_sum(sT[:, g:g + 1], xt[:], axis=AX.X)
                    xts.append(xt)
                # raw sums -> means
                nc.vector.tensor_scalar(
                    out=sT[:], in0=sT[:], scalar1=1.0 / HWPIX, scalar2=None,
                    op0=Alu.mult,
                )

                # ---- KAN layer 1: s (512,) -> h1 (64,) ----
                silu1 = spool.tile([128, NG], f32, tag="silu1")
                nc.scalar.activation(silu1[:], sT[:], Act.Silu)
                bf = spool.tile([128, NG * KB], f32, tag="bf")
                for g in range(NG):
                    _emit_bsplines(
                        nc, mybir, bpool, gt1_sb, sT[:, g:g + 1],
                        bf[:, KB * g:KB * (g + 1)], 128, rs1,
                    )
                ps1 = ppool.tile([HIDDEN, 1], f32, tag="ps1")
                mms = []
                for g in range(NG):
                    mms.append((w1t_sb[:, HIDDEN * g:HIDDEN * (g + 1)], silu1[:, g:g + 1]))
                for g in range(NG):
                    for k in range(KB):
                        col = HIDDEN * (KB * g + k)
                        mms.append((sw1_sb[:, col:col + HIDDEN], bf[:, KB * g + k:KB * g + k + 1]))
                for i, (lhsT, rhs) in enumerate(mms):
                    nc.tensor.matmul(
                        ps1[:], lhsT, rhs, start=(i == 0), stop=(i == len(mms) - 1)
                    )

                # ---- inter-layer SiLU, KAN layer 2: t (64,) -> (512,) ----
                t1 = spool.tile([HIDDEN, 1], f32, tag="t1")
                nc.scalar.activation(t1[:], ps1[:], Act.Silu)
                silu2 = spool.tile([HIDDEN, 1], f32, tag="silu2")
                nc.scalar.activation(silu2[:], t1[:], Act.Silu)
                b2f = spool.tile([HIDDEN, KB], f32, tag="b2f")
                _emit_bsplines(nc, mybir, bpool, gt2_sb, t1[:, 0:1], b2f[:], HIDDEN, rs2)

                ps2 = ppool.tile([128, NG], f32, tag="ps2")
                for og in range(NG):
                    mms2 = [(w2t_sb[:, 128 * og:128 * (og + 1)], silu2[:, 0:1])]
                    for k in range(KB):
                        col = C * k + 128 * og
                        mms2.append((sw2_sb[:, col:col + 128], b2f[:, k:k + 1]))
                    for i, (lhsT, rhs) in enumerate(mms2):
                        nc.tensor.matmul(
                            ps2[:, og:og + 1], lhsT, rhs,
                            start=(i == 0), stop=(i == len(mms2) - 1),
                        )

                gate = spool.tile([128, NG], f32, tag="gate")
                nc.scalar.activation(gate[:], ps2[:], Act.Sigmoid)

                # ---- scale resident tiles by the gate, store ----
                # Scale on ScalarE (activation Copy with per-partition scale) and
                # store on the ACT HWDGE ring: the SP ring then carries only loads,
                # so a store waiting on the gate can never head-of-line-block the
                # next sample's loads (HWDGE DMAs execute FIFO per issuing engine).
                for g in range(NG):
                    nc.scalar.activation(
                        xts[g][:], xts[g][:], Act.Copy, scale=gate[:, g:g + 1],
                    )
                    dst = y_d[n, 128 * g:128 * (g + 1)].rearrange("p h w -> p (h w)")
                    nc.scalar.dma_start(dst, xts[g][:])
    nc.compile()
    return nc


def _run(inputs, trace=False):
    from concourse.bass_utils import run_bass_kernel_spmd

    x = np.ascontiguousarray(np.asarray(inputs["x"], np.float32))
    assert x.shape == (B, C, H, W), x.shape
    tensors, rs1, rs2 = _host_prep(inputs)
    nc = _build_nc(rs1, rs2)
    in_maps = []
    for c in range(NCORES):
        m = {"x": np.ascontiguousarray(x[NS * c:NS * (c + 1)])}
        m.update(tensors)
        in_maps.append(m)
    res = run_bass_kernel_spmd(
        nc, in_maps, core_ids=list(range(NCORES)), trace=trace
    )
    out = np.concatenate([res.results[c]["y"] for c in range(NCORES)], axis=0)
    return out, res


def kernel(**inputs) -> np.ndarray:
    return _run(inputs)[0]



# revision 3
# speedup vs baseline: 1.4503x; 1.4503x over previous
"""KAN-SE (squeeze-excite with 2-layer KAN MLP) Trainium2 kernel.

Full-input contract: kernel(**inputs) takes the complete (32, 512, 64, 64)
batch plus KAN weights, shards the batch across 8 NeuronCores (4 samples
per core, data-parallel, weights replicated), and returns the full output.

Per-core device program (pure SPMD, no collectives), per sample:
  - load the sample's (512, 4096) pixels as 4 tiles of (128, 4096) f32 on
    the SP HWDGE ring, keep them resident in SBUF
  - per-channel mean via free-dim reduce (VectorE)
  - 2-layer KAN on the means:
      * cubic B-spline bases via Cox-de-Boor on VectorE, with all 4
        channel groups merged per op through stride-0 (broadcast) APs
      * einsums as bf16 PE matmuls accumulating in f32 PSUM; layer-2
        contracts (hidden, k) pairs on 128 partitions via a shifted-knot
        duplicate of the basis computation (20 matmuls instead of 36)
      * SiLU as x*sigmoid(x) so ScalarE only ever loads the Sigmoid table
  - scale the resident tiles by the gate (VectorE), store on the ACT
    HWDGE ring (so gate-waiting stores never head-of-line-block the SP
    ring's loads for the next sample)

x is read exactly once (SBUF-resident between mean and scale), so HBM
traffic is the 2x minimum: 8 MiB in + 8 MiB out per sample per core.
"""

import numpy as np
import ml_dtypes

BF16 = ml_dtypes.bfloat16

# ---- problem constants (hardcoded per contract; do not read spec/reference) ----
B, C, H, W = 32, 512, 64, 64
HIDDEN = 64            # max(16, 512 // 8)
KB = 8                 # GRID_SIZE + SPLINE_ORDER = 5 + 3
NCORES = 8
NS = B // NCORES       # samples per core = 4
NG = C // 128          # channel groups of 128 = 4
HWPIX = H * W          # 4096

# gtab column layout: [g_i(12) | -g_i*rs1(10) | g_{i+2}*rs1(10)
#                      | -g_i*rs2(9) | g_{i+3}*rs2(9) | -g_i*rs3(8) | g_{i+4}*rs3(8)]
_GT_OFF = {1: (12, 22), 2: (32, 41), 3: (50, 58)}
_GT_W = 66


def _grid_table_rows(g):
    """One (66,) row of prescaled Cox-de-Boor constants from 12 knots."""
    g = np.asarray(g, np.float64)
    assert g.shape == (12,)
    h = g[1] - g[0]
    tab = np.zeros((_GT_W,), np.float64)
    tab[0:12] = g
    for k in (1, 2, 3):
        w = 11 - k
        rs = 1.0 / (k * h)
        aoff, coff = _GT_OFF[k]
        tab[aoff:aoff + w] = -g[:w] * rs          # -g_i / (k h)
        tab[coff:coff + w] = g[k + 1:12] * rs     # g_{i+k+1} / (k h)
    return tab.astype(np.float32), float(h)


def _host_prep(inputs):
    """Rearrange weights into the SBUF layouts the device program uses."""
    f32 = np.float32
    bw1 = np.asarray(inputs["base_w1"], f32)                      # (64, 512)
    sw1 = (np.asarray(inputs["spline_w1"], f32)
           * np.asarray(inputs["scaler1"], f32)[:, :, None])      # (64, 512, 8)
    bw2 = np.asarray(inputs["base_w2"], f32)                      # (512, 64)
    sw2 = (np.asarray(inputs["spline_w2"], f32)
           * np.asarray(inputs["scaler2"], f32)[:, :, None])      # (512, 64, 8)

    # Layer 1: 36 lhsT blocks of (128, 128), rhs col order [g]*9 + j
    # (j=0 base, j=1+k spline).  Output columns are the 64 hidden units
    # duplicated twice so ps1 comes out as (128, 1) = h1 stacked [h1; h1],
    # which layer 2's shifted-knot basis trick needs.
    blocks = []
    for g in range(NG):
        bT = bw1[:, 128 * g:128 * (g + 1)].T                      # (128, 64) [p, o]
        blocks.append(np.concatenate([bT, bT], axis=1))
        for k in range(KB):
            sT = sw1[:, 128 * g:128 * (g + 1), k].T               # (128, 64)
            blocks.append(np.concatenate([sT, sT], axis=1))
    l1w = np.concatenate(blocks, axis=1)                          # (128, 4608)

    # Layer 2: 20 lhsT blocks of (128, 128): per out-group, [base | c=0,2,4,6].
    # Partition rows 0-63 contract hidden-unit i with basis k=c; rows 64-127
    # contract the same i with k=c+1 (the basis tile's rows 64-127 hold
    # B_{k+1}(t1[i]) via the shifted grid table).
    blocks2 = []
    z64 = np.zeros((64, 128), f32)
    for og in range(NG):
        bT = bw2[128 * og:128 * (og + 1), :].T                    # (64, 128) [i, o]
        blocks2.append(np.concatenate([bT, z64], axis=0))
        for c in (0, 2, 4, 6):
            top = sw2[128 * og:128 * (og + 1), :, c].T            # (64, 128)
            bot = sw2[128 * og:128 * (og + 1), :, c + 1].T
            blocks2.append(np.concatenate([top, bot], axis=0))
    l2w = np.concatenate(blocks2, axis=1)                         # (128, 2560)

    g1 = np.asarray(inputs["grid1"], f32)[0]
    g2 = np.asarray(inputs["grid2"], f32)[0]
    row1, h1 = _grid_table_rows(g1)
    gt1 = np.tile(row1[None, :], (128, 1))
    # gt2: rows 0-63 from the true knots; rows 64-127 from knots shifted by
    # one (g_1..g_12 with g_12 extrapolated), so those rows' basis k is the
    # true basis k+1.
    row2a, h2 = _grid_table_rows(g2)
    g2s = np.concatenate([np.asarray(g2, np.float64)[1:],
                          [np.asarray(g2, np.float64)[11] + (g2[1] - g2[0])]])
    row2b, _ = _grid_table_rows(g2s)
    gt2 = np.concatenate([np.tile(row2a[None, :], (64, 1)),
                          np.tile(row2b[None, :], (64, 1))], axis=0)

    tensors = {
        "l1w": np.ascontiguousarray(l1w.astype(BF16)),
        "l2w": np.ascontiguousarray(l2w.astype(BF16)),
        "gt1": np.ascontiguousarray(gt1, f32),
        "gt2": np.ascontiguousarray(gt2, f32),
    }
    rs1 = {k: float(np.float32(1.0 / (k * h1))) for k in (1, 2, 3)}
    rs2 = {k: float(np.float32(1.0 / (k * h2))) for k in (1, 2, 3)}
    return tensors, rs1, rs2


def _emit_chain(nc, mybir, pool, gt_sb, x_col, out3, ngrp, rs):
    """Cubic B-spline bases (Cox-de-Boor) for ngrp groups at once.

    x_col: (128, ngrp) AP, one x per (partition, group).
    gt_sb: (128, 66) constant table (shared across groups via stride-0 dims).
    out3:  (128, ngrp, 8) strided output AP.
    rs:    {k: 1/(k*h)} reciprocal immediates.
    """
    f32 = mybir.dt.float32
    Alu = mybir.AluOpType
    P = 128

    def gtv(lo, w):
        return gt_sb[:, lo:lo + w].unsqueeze(1).to_broadcast((P, ngrp, w))

    def xv(w):
        return x_col.unsqueeze(2).to_broadcast((P, ngrp, w))

    ge_t = pool.tile([P, ngrp * 12], f32, tag="ge", bufs=4)
    ge = ge_t[:].rearrange("p (g w) -> p g w", g=ngrp)
    nc.vector.tensor_tensor(ge, gtv(0, 12), xv(12), Alu.is_le)
    b_t = pool.tile([P, ngrp * 11], f32, tag="b0", bufs=4)
    bprev = b_t[:].rearrange("p (g w) -> p g w", g=ngrp)
    nc.vector.tensor_tensor(bprev, ge[:, :, 0:11], ge[:, :, 1:12], Alu.subtract)
    for k in (1, 2, 3):
        w = 11 - k
        aoff, coff = _GT_OFF[k]
        u_t = pool.tile([P, ngrp * 10], f32, tag="u", bufs=4)
        a_t = pool.tile([P, ngrp * 10], f32, tag="a", bufs=4)
        c_t = pool.tile([P, ngrp * 10], f32, tag="c", bufs=4)
        u = u_t[:].rearrange("p (g w) -> p g w", g=ngrp)[:, :, :w]
        a = a_t[:].rearrange("p (g w) -> p g w", g=ngrp)[:, :, :w]
        c = c_t[:].rearrange("p (g w) -> p g w", g=ngrp)[:, :, :w]
        # u = x/(k h);  A = u - g_i/(k h);  C = g_{i+k+1}/(k h) - u
        nc.vector.tensor_scalar(
            out=u, in0=xv(w), scalar1=rs[k], scalar2=None, op0=Alu.mult
        )
        nc.vector.tensor_tensor(a, u, gtv(aoff, w), Alu.add)
        nc.vector.tensor_tensor(c, gtv(coff, w), u, Alu.subtract)
        if k < 3:
            bn_t = pool.tile([P, ngrp * 10], f32, tag="bn", bufs=4)
            outp = bn_t[:].rearrange("p (g w) -> p g w", g=ngrp)[:, :, :w]
        else:
            outp = out3
        nc.vector.tensor_tensor(c, c, bprev[:, :, 1:w + 1], Alu.mult)
        nc.vector.tensor_tensor(outp, a, bprev[:, :, 0:w], Alu.mult)
        nc.vector.tensor_tensor(outp, outp, c, Alu.add)
        if k < 3:
            bprev = outp


def _build_nc(rs1, rs2):
    import concourse.bacc as bacc
    import concourse.bass as bass  # noqa: F401
    import concourse.mybir as mybir
    from concourse.tile import TileContext

    f32 = mybir.dt.float32
    bf16 = mybir.dt.bfloat16
    Alu = mybir.AluOpType
    Act = mybir.ActivationFunctionType
    AX = mybir.AxisListType

    # Bacc (not plain Bass): its compile() runs move_matmul_waits_to_ldweights
    # + generate_event_semaphores, which split multi-waits down to the 1-wait-
    # per-instruction TRN2 ISA limit that walrus enforces.
    nc = bacc.Bacc("TRN2", target_bir_lowering=False)
    x_d = nc.declare_dram_parameter("x", [NS, C, H, W], f32, isOutput=False)
    l1w_d = nc.declare_dram_parameter("l1w", [128, 36 * 128], bf16, isOutput=False)
    l2w_d = nc.declare_dram_parameter("l2w", [128, 20 * 128], bf16, isOutput=False)
    gt1_d = nc.declare_dram_parameter("gt1", [128, _GT_W], f32, isOutput=False)
    gt2_d = nc.declare_dram_parameter("gt2", [128, _GT_W], f32, isOutput=False)
    y_d = nc.declare_dram_parameter("y", [NS, C, H, W], f32, isOutput=True)

    with TileContext(nc) as tc:
        with (
            tc.tile_pool(name="consts", bufs=1) as cpool,
            tc.tile_pool(name="xdata", bufs=10) as xpool,
            tc.tile_pool(name="small", bufs=3) as spool,
            tc.tile_pool(name="bspl", bufs=1) as bpool,
            tc.tile_pool(name="psum", bufs=2, space="PSUM") as ppool,
        ):
            l1w_sb = cpool.tile([128, 36 * 128], bf16)
            nc.sync.dma_start(l1w_sb[:], l1w_d[:, :])
            l2w_sb = cpool.tile([128, 20 * 128], bf16)
            nc.sync.dma_start(l2w_sb[:], l2w_d[:, :])
            gt1_sb = cpool.tile([128, _GT_W], f32)
            nc.sync.dma_start(gt1_sb[:], gt1_d[:, :])
            gt2_sb = cpool.tile([128, _GT_W], f32)
            nc.sync.dma_start(gt2_sb[:], gt2_d[:, :])

            # Pre-touch every const tile: the DMA-completion wait lands on
            # these throwaway ops, so later consumers (whose ISA formats have
            # a single wait slot) never need a DMA wait of their own.
            touch = cpool.tile([128, 4], f32)
            for i, ct in enumerate((gt1_sb, gt2_sb)):
                nc.vector.tensor_copy(touch[:, i:i + 1], ct[:, 0:1])
            pt_ps = ppool.tile([1, 2], f32, tag="pt")
            for i, ct in enumerate((l1w_sb, l2w_sb)):
                nc.tensor.matmul(pt_ps[0:1, i:i + 1], ct[:, 0:1], ct[:, 0:1],
                                 start=True, stop=True)

            for n in range(NS):
                # ---- load sample, per-channel sums ----
                sums = spool.tile([128, NG], f32, tag="sums")
                xts = []
                for g in range(NG):
                    xt = xpool.tile([128, HWPIX], f32, tag="xt")
                    src = x_d[n, 128 * g:128 * (g + 1)].rearrange("p h w -> p (h w)")
                    nc.sync.dma_start(xt[:], src)
                    nc.vector.reduce_sum(sums[:, g:g + 1], xt[:], axis=AX.X)
                    xts.append(xt)
                means = spool.tile([128, NG], f32, tag="means")
                nc.vector.tensor_scalar(
                    out=means[:], in0=sums[:], scalar1=1.0 / HWPIX, scalar2=None,
                    op0=Alu.mult,
                )

                # ---- KAN layer 1: s (512,) -> h1 (64, duplicated to 128) ----
                sigS = spool.tile([128, NG], f32, tag="sigS")
                nc.scalar.activation(sigS[:], means[:], Act.Sigmoid)
                rhs1f = spool.tile([128, NG * 9], f32, tag="rhs1f")
                r1v = rhs1f[:].rearrange("p (g w) -> p g w", g=NG)
                # silu(s) = s * sigmoid(s) into column 0 of each group's block
                nc.vector.tensor_tensor(
                    r1v[:, :, 0:1], sigS[:].unsqueeze(2), means[:].unsqueeze(2),
                    Alu.mult,
                )
                _emit_chain(nc, mybir, bpool, gt1_sb, means[:], r1v[:, :, 1:9],
                            NG, rs1)
                rhs1b = spool.tile([128, NG * 9], bf16, tag="rhs1b")
                nc.vector.tensor_copy(rhs1b[:], rhs1f[:])

                ps1 = ppool.tile([128, 1], f32, tag="ps1")
                for i in range(36):
                    nc.tensor.matmul(
                        ps1[:], l1w_sb[:, 128 * i:128 * (i + 1)],
                        rhs1b[:, i:i + 1], start=(i == 0), stop=(i == 35),
                    )

                # ---- inter-layer SiLU, KAN layer 2: t (128-dup) -> (512,) ----
                sig1 = spool.tile([128, 1], f32, tag="sig1")
                nc.scalar.activation(sig1[:], ps1[:], Act.Sigmoid)
                t1 = spool.tile([128, 1], f32, tag="t1")
                nc.vector.tensor_tensor(t1[:], ps1[:], sig1[:], Alu.mult)
                sig2 = spool.tile([128, 1], f32, tag="sig2")
                nc.scalar.activation(sig2[:], t1[:], Act.Sigmoid)
                rhs2f = spool.tile([128, 9], f32, tag="rhs2f")
                nc.vector.tensor_tensor(rhs2f[:, 0:1], t1[:], sig2[:], Alu.mult)
                _emit_chain(nc, mybir, bpool, gt2_sb, t1[:],
                            rhs2f[:].unsqueeze(1)[:, :, 1:9], 1, rs2)
                rhs2b = spool.tile([128, 9], bf16, tag="rhs2b")
                nc.vector.tensor_copy(rhs2b[:], rhs2f[:])

                ps2 = ppool.tile([128, NG], f32, tag="ps2")
                for og in range(NG):
                    i0 = 5 * og
                    nc.tensor.matmul(
                        ps2[:, og:og + 1], l2w_sb[:, 128 * i0:128 * (i0 + 1)],
                        rhs2b[:, 0:1], start=True, stop=False,
                    )
                    for ci, cc in enumerate((0, 2, 4, 6)):
                        i = i0 + 1 + ci
                        nc.tensor.matmul(
                            ps2[:, og:og + 1], l2w_sb[:, 128 * i:128 * (i + 1)],
                            rhs2b[:, 1 + cc:2 + cc], start=False, stop=(ci == 3),
                        )

                gate = spool.tile([128, NG], f32, tag="gate")
                nc.scalar.activation(gate[:], ps2[:], Act.Sigmoid)

                # ---- scale resident tiles by the gate, store on ACT ring ----
                for g in range(NG):
                    nc.vector.tensor_scalar(
                        out=xts[g][:], in0=xts[g][:], scalar1=gate[:, g:g + 1],
                        scalar2=None, op0=Alu.mult,
                    )
                    dst = y_d[n, 128 * g:128 * (g + 1)].rearrange("p h w -> p (h w)")
                    nc.scalar.dma_start(dst, xts[g][:])
    nc.compile()
    return nc


def _run(inputs, trace=False):
    from concourse.bass_utils import run_bass_kernel_spmd

    x = np.ascontiguousarray(np.asarray(inputs["x"], np.float32))
    assert x.shape == (B, C, H, W), x.shape
    tensors, rs1, rs2 = _host_prep(inputs)
    nc = _build_nc(rs1, rs2)
    in_maps = []
    for c in range(NCORES):
        m = {"x": np.ascontiguousarray(x[NS * c:NS * (c + 1)])}
        m.update(tensors)
        in_maps.append(m)
    res = run_bass_kernel_spmd(
        nc, in_maps, core_ids=list(range(NCORES)), trace=trace
    )
    out = np.concatenate([res.results[c]["y"] for c in range(NCORES)], axis=0)
    return out, res


def kernel(**inputs) -> np.ndarray:
    return _run(inputs)[0]
